# revision 31
# baseline (speedup 1.0000x reference)
"""Causal multi-head self-attention (B=2, S=2048, D=768, H=12) on 8 TRN2 NeuronCores.

Sharding: core c = (batch b=c//4, head-group hg=c%4 of 3 heads).
Each core computes Q/K/V for its 3 heads, causal attention, and the partial
output projection sum_h out_h @ Wo[:, h]^T -> (S, D). Host sums the 4
head-group partials per batch (the unshard step).

v2 restructure (from trace analysis of the v1 baseline @171us):
  - Input DMA issue is serialized ~730ns per dma_start on the issuing
    engine; v1's 45 fine-grained input DMAs stretched input arrival to
    ~40us. Now ~12 coarse DMAs split across BOTH HWDGE queues (sync +
    scalar), consumption-ordered: input fully lands by ~20us.
  - No warmup chain (v1's 120-mm chain blocked the A stream until 23.8us
    behind a PSUM bank conflict). A starts as soon as wcat m0 + xt sh0/1
    land; the stream itself warms the HAM clock gate while DMA-paced.
  - Single TileContext scope (v1's scope barrier dropped the HAM to
    1.2GHz for 8.2us at the scope-1/2 boundary).
  - Out-projection (D) is split into per-qtile 2x384-col pieces and
    interleaved into the last C block + its tail, so the output DMA
    (issued on sync) overlaps compute instead of forming a 14us tail.
  - A/tr/D work is fed into C-block exp-wait slots as filler closures
    (engine queues are in-order; interleaving must happen at program
    order level).

HAM clock-gate: full 128x128-stationary matmuls everywhere (partial-array
ones don't count as PE-busy and let the clock gate fall to 1.2GHz):
  - scores: stationary = the full 128-row qkvt chunk (the cohabitant
    head's rows are neutralized by a zero-padded Q moving operand qz)
  - PV: stationary = vp = [V | ones | zeros] padded to 128 cols
  - out-proj: oct rows 64-127 of chunk 1 zeroed, stationary full-height
All matmul operands bf16; output written bf16 (host upcasts).

C per head, per q-half qp (2 q-chunks of 512), per k-tile t:
  scoresT[k, q] = Kchunk.T @ qz   (only causally-valid halves)
  additive -30000 mask on the diagonal half, exp on ACT -> bf16,
  PV: pout[qc] += vp[t].T @ expT  (one vp column of ones = denominator;
  h1's V occupies vp cols 64:128 so its numerator lands partition-aligned
  with its oct_ slot)
then per qc: DMA-spread den -> 64-lane recip -> gather -> gpsimd
broadcast -> partition-aligned multiply into outcatT.
D piece: psum[q, j] += outcatT[:, q].T @ WoT[:, j]; copy; DMA out.
"""

import numpy as np
import ml_dtypes
from collections import deque
from contextlib import ExitStack

import concourse.bass as bass
import concourse.tile as tile
from concourse import bacc, mybir
from concourse import bass_utils

F32 = mybir.dt.float32
BF16 = mybir.dt.bfloat16
AF = mybir.ActivationFunctionType
BF = ml_dtypes.bfloat16

B, S, D, H = 2, 2048, 768, 12
DK = 64
HPC = 3            # heads per core
NCORES = 8
NI = D // 128      # 6 input-feature chunks
NM = 5             # output m-chunks of 128 (640 rows incl. 64 pad)
NT = S // 128      # 16 k-tiles
NQC = S // 512     # 4 q-chunks
MASK_NEG = -30000.0

# wcat m-chunks: m0=[q0;q1] m1=[k0;k1] m2=[q2;v0] m3=[k2;v1] m4=[v2;pad]
KCHUNK = [1, 1, 3]   # score stationary = full 128-row chunk holding K_h
VPOS = [(64, 2), (64, 3), (0, 4)]
VCOL = [0, 0, 0]     # V dst col base in vp
NR0 = [0, 0, 0]      # PV numerator psum row base
DROW = [64, 64, 64]  # PV denominator psum row

_NC_CACHE = {}


def build_nc():
    if "nc" in _NC_CACHE:
        return _NC_CACHE["nc"]
    nc = bacc.Bacc("TRN2", target_bir_lowering=False, debug=False,
                   num_devices=NCORES)

    xt_d = nc.dram_tensor("xt", [NI, 128, S], BF16, kind="ExternalInput").ap()
    wcat_d = nc.dram_tensor("wcat", [NI, 128, NM * 128], BF16, kind="ExternalInput").ap()
    wot_d = nc.dram_tensor("wot", [2, 128, D], BF16, kind="ExternalInput").ap()
    mask_d = nc.dram_tensor("mask", [128, 128], F32, kind="ExternalInput").ap()
    id_d = nc.dram_tensor("ident", [128, 128], BF16, kind="ExternalInput").ap()
    out_d = nc.dram_tensor("out", [S, D], BF16, kind="ExternalOutput").ap()

    with tile.TileContext(nc) as tc, ExitStack() as ctx:
        const = ctx.enter_context(tc.tile_pool(name="const", bufs=1))

        # persistent SBUF buffers
        xt = const.tile([128, NI, S], BF16)             # X^T
        wcat = const.tile([128, NI, NM * 128], BF16)    # W^T (QKV packed)
        wot = const.tile([128, 2, D], BF16)             # Wo^T [h0;h1],[h2;0]
        maskb = const.tile([128, 128], F32)             # diag causal bias tile
        ident = const.tile([128, 128], BF16)
        qkvt = const.tile([128, NM, S], BF16)           # K^T/V^T chunks
        qz = const.tile([128, HPC, S], BF16)            # zero-padded Q^T per head
        vp = const.tile([128, HPC, NT, 128], BF16)      # V' = [V | ones | 0]
        oct_ = const.tile([128, 2, S], BF16)            # packed out^T [h0;h1],[h2;0]

        # ---- coarse input DMAs, consumption-ordered, split across the two
        # HWDGE issue queues (sync + scalar) so descriptor pushes (~730ns
        # each, serialized per engine) don't gate data arrival. A single
        # DMA only sustains ~100-150 GB/s, so each xt seq-block is split
        # into two half-depth DMAs, one per queue, to land ~2x faster.
        def dma_wcat(eng, m):
            eng.dma_start(wcat[:, :, m * 128:(m + 1) * 128],
                          wcat_d[:, :, m * 128:(m + 1) * 128]
                          .rearrange("c p f -> p c f"))

        def dma_xt(eng, sh, ih):
            i0, i1 = 3 * ih, 3 * ih + 3
            eng.dma_start(xt[:, i0:i1, sh * 512:(sh + 1) * 512],
                          xt_d[i0:i1, :, sh * 512:(sh + 1) * 512]
                          .rearrange("c p f -> p c f"))

        dma_wcat(nc.sync, 0)
        # first xt piece split small so the very first matmul can start
        # ~2-3us earlier while the HAM is still cold anyway
        nc.scalar.dma_start(xt[:, 0:1, 0:512],
                            xt_d[0:1, :, 0:512].rearrange("c p f -> p c f"))
        nc.scalar.dma_start(xt[:, 1:3, 0:512],
                            xt_d[1:3, :, 0:512].rearrange("c p f -> p c f"))
        dma_xt(nc.sync, 0, 1)
        nc.scalar.dma_start(ident[:], id_d)
        dma_xt(nc.sync, 1, 1)
        dma_xt(nc.scalar, 1, 0)
        dma_wcat(nc.sync, 2)
        dma_wcat(nc.scalar, 1)
        nc.sync.dma_start(maskb[:], mask_d)
        dma_xt(nc.scalar, 2, 0)
        dma_xt(nc.sync, 2, 1)
        dma_xt(nc.scalar, 3, 0)
        dma_xt(nc.sync, 3, 1)
        dma_wcat(nc.scalar, 3)
        dma_wcat(nc.sync, 4)
        nc.scalar.dma_start(wot[:], wot_d.rearrange("c p f -> p c f"))

        # zero/one fills on DVE, all disjoint from later writers (the vp pad
        # init deliberately avoids cols 0:DK so the V transposes never wait)
        nc.vector.memzero(qz[64:128, 0, :])
        nc.vector.memzero(qz[0:64, 1, :])
        nc.vector.memzero(qz[64:128, 2, :])
        nc.vector.memzero(oct_[64:128, 1, :])
        nc.vector.memzero(vp[:, :, :, DK:128])         # pad cols
        nc.vector.memset(vp[:, :, :, DK:DK + 1], 1.0)  # denominator ones col


        sb_exp = ctx.enter_context(tc.tile_pool(name="sb_exp", bufs=6))
        sb_div = ctx.enter_context(tc.tile_pool(name="sb_div", bufs=3))

        ps_f = ctx.enter_context(tc.tile_pool(name="ps_f", bufs=2, space="PSUM"))
        ps_s = ctx.enter_context(tc.tile_pool(name="ps_s", bufs=2, space="PSUM"))
        ps_o = ctx.enter_context(tc.tile_pool(name="ps_o", bufs=2, space="PSUM"))

        def a_closures(m, scp):
            """QKV projection for m-chunk m, seq half scp (2x512 cols), as
            one closure per contraction chunk i (2 matmuls each)."""
            pqs = [None, None]

            def step(i):
                for half in range(2):
                    sc = 2 * scp + half
                    if i == 0:
                        pqs[half] = ps_f.tile([128, 512], F32, tag="fill",
                                              name=f"pq{m}_{2 * scp + half}")
                    nc.tensor.matmul(
                        pqs[half][:],
                        wcat[:, i, m * 128:(m + 1) * 128],
                        xt[:, i, sc * 512:(sc + 1) * 512],
                        start=(i == 0), stop=(i == NI - 1))
                if i != NI - 1:
                    return
                for half in range(2):
                    sc = 2 * scp + half
                    s0, s1 = sc * 512, (sc + 1) * 512
                    pq = pqs[half]
                    # alternate the PSUM-evict engine per half so copies
                    # run two-wide (a single engine's ~680ns/copy chain
                    # gates the transposes/scores that consume qkvt)
                    cp = nc.vector.tensor_copy if half == 0 else nc.scalar.copy
                    if m == 0:      # pure Q chunk -> zero-padded q shadows
                        cp(qz[0:64, 0, s0:s1], pq[0:64, :])
                        cp(qz[64:128, 1, s0:s1], pq[64:128, :])
                    elif m == 2:    # [q2; v0]
                        cp(qz[0:64, 2, s0:s1], pq[0:64, :])
                        cp(qkvt[64:128, m, s0:s1], pq[64:128, :])
                    elif m == 4:    # [v2; pad]
                        cp(qkvt[0:64, m, s0:s1], pq[0:64, :])
                    else:           # full K chunks (score stationaries)
                        cp(qkvt[:, m, s0:s1], pq[:])

            return [(lambda i=i: step(i)) for i in range(NI)]

        def tr_closures(h, tlo, thi):
            """V transposes into vp, 4 k-tiles per closure."""
            vb, vchunk = VPOS[h]
            vc = VCOL[h]

            def step(base):
                for t in range(base, min(base + 4, thi)):
                    ptr = ps_f.tile([128, DK], BF16, tag="fill",
                                    name=f"tr{h}_{t}")
                    nc.tensor.transpose(
                        ptr[:], qkvt[vb:vb + DK, vchunk, t * 128:(t + 1) * 128],
                        ident[vb:vb + DK, vb:vb + DK])
                    nc.vector.tensor_copy(vp[:, h, t, vc:vc + DK], ptr[:])

            return [(lambda b=b: step(b)) for b in range(tlo, thi, 4)]

        def d_closures(qts, act_copy=False):
            """Out-projection pieces: per q-tile, 2 closures of a 2x384-col
            matmul pair + copy; the second closure DMAs the tile out."""
            cls = []
            for qt in qts:
                oref = [None]

                def piece(qt, p, oref):
                    if p == 0:
                        oref[0] = sb_exp.tile([128, D], BF16, tag="exp",
                                              name=f"ot{qt}")
                    pp = ps_f.tile([128, 384], F32, tag="fill",
                                   name=f"pp{qt}_{p}")
                    for c in (0, 1):
                        nc.tensor.matmul(
                            pp[:],
                            oct_[:, c, qt * 128:(qt + 1) * 128],
                            wot[:, c, p * 384:(p + 1) * 384],
                            start=(c == 0), stop=(c == 1))
                    # alternate engines so the tail copy drain runs 2-wide
                    use_act = (qt + p) % 2 == 0 if act_copy else False
                    if use_act:
                        nc.scalar.copy(oref[0][:, p * 384:(p + 1) * 384], pp[:])
                    else:
                        nc.vector.tensor_copy(
                            oref[0][:, p * 384:(p + 1) * 384], pp[:])
                    if p == 1:
                        nc.sync.dma_start(
                            out_d[qt * 128:(qt + 1) * 128, :], oref[0][:])

                cls.append(lambda qt=qt, o=oref: piece(qt, 0, o))
                cls.append(lambda qt=qt, o=oref: piece(qt, 1, o))
            return cls

        def c_block(h, qp, fillers=(), post_div_fillers=()):
            kchunk = KCHUNK[h]
            pouts = {}
            fq = deque(fillers)
            pdq = deque(post_div_fillers)

            def score_step(t):
                qcs = (2 * qp, 2 * qp + 1)
                qc_lo = t // 4
                off = 128 * (t % 4)   # diag col offset inside qc_lo's half
                pscr = ps_s.tile([128, 1024], F32, tag="scr",
                                 name=f"sc{h}_{qp}_{t}")
                for half, qc in enumerate(qcs):
                    if qc < qc_lo:
                        continue
                    cs = off if qc == qc_lo else 0  # skip fully-masked cols
                    nc.tensor.matmul(
                        pscr[:, half * 512 + cs:(half + 1) * 512],
                        qkvt[:, kchunk, t * 128:(t + 1) * 128],
                        qz[:, h, qc * 512 + cs:(qc + 1) * 512],
                        start=True, stop=True)
                if qc_lo in qcs:  # mask only the 128-wide diagonal window
                    half = qc_lo - 2 * qp
                    nc.vector.tensor_add(
                        pscr[:, half * 512 + off:half * 512 + off + 128],
                        pscr[:, half * 512 + off:half * 512 + off + 128],
                        maskb[:, 0:128])
                lo = (512 if qc_lo == qcs[1] else 0) + \
                     (off if qc_lo in qcs else 0)
                expt = sb_exp.tile([128, 1024], BF16, tag="exp",
                                   name=f"ex{h}_{qp}_{t}")
                nc.scalar.activation(expt[:, lo:1024], pscr[:, lo:1024],
                                     AF.Exp)
                return expt

            def pv_step(t, expt):
                qcs = (2 * qp, 2 * qp + 1)
                qc_lo = t // 4
                off = 128 * (t % 4)
                for half, qc in enumerate(qcs):
                    if qc < qc_lo:
                        continue
                    cs = off if qc == qc_lo else 0
                    nc.tensor.matmul(
                        pouts[qc][:, cs:512],
                        vp[:, h, t, :],
                        expt[:, half * 512 + cs:(half + 1) * 512],
                        start=(t == 0), stop=(t == 4 * qc + 3))

            def divide(qc):
                # evict the finished chain so its PSUM bank frees; the
                # recip runs 64 lanes wide on a DMA-spread [64,8] view of
                # the den row (a [64,512]-wide reciprocal is ~6.5ns/elem on
                # DVE -- measured 3.3us -- so keep recip elements minimal).
                nout = sb_div.tile([DK + 1, 512], F32, tag="nout",
                                   name=f"no{h}_{qc}")
                nc.vector.tensor_copy(nout[:], pouts[qc][0:DK + 1, :])
                rsp = sb_div.tile([DK, 8], F32, tag="rsp", name=f"rsp{h}_{qc}")
                nc.sync.dma_start(rsp[:], nout[DK:DK + 1, :])
                rcs = sb_div.tile([DK, 8], F32, tag="rcs", name=f"rcs{h}_{qc}")
                nc.vector.reciprocal(rcs[:], rsp[:])
                rc0 = sb_div.tile([1, 512], F32, tag="rc0", name=f"rc0{h}_{qc}")
                nc.sync.dma_start(rc0[:], rcs[:])
                rb = sb_div.tile([DK, 512], F32, tag="rb", name=f"rb{h}_{qc}")
                nc.gpsimd.partition_broadcast(rb[:], rc0[:])
                if h == 1:
                    # h1 lands at partitions 64-127: shift via SBUF DMA
                    tmp = sb_div.tile([DK, 512], BF16, tag="tmp",
                                      name=f"tmp{h}_{qc}")
                    nc.vector.tensor_mul(tmp[:], nout[0:DK, :], rb[:])
                    nc.sync.dma_start(
                        oct_[DK:128, 0, qc * 512:(qc + 1) * 512], tmp[:])
                else:
                    nc.vector.tensor_mul(
                        oct_[0:DK, h // 2, qc * 512:(qc + 1) * 512],
                        nout[0:DK, :], rb[:])

            for qc in (2 * qp, 2 * qp + 1):
                pouts[qc] = ps_o.tile([128, 512], F32, tag="pout",
                                      name=f"po{h}_{qc}")
            t_pairs = list(range(0, 4 * (2 * qp + 1) + 4, 2))
            for pi, t0 in enumerate(t_pairs):
                e0 = score_step(t0)
                e1 = score_step(t0 + 1)
                # feed independent work into the exp-wait slot
                iters_left = len(t_pairs) - pi
                for _ in range(-(-len(fq) // iters_left)):
                    if fq:
                        fq.popleft()()
                pv_step(t0, e0)
                pv_step(t0 + 1, e1)
                if t0 + 1 == 4 * (2 * qp) + 3:
                    divide(2 * qp)      # low chain done: free its bank
                    fq.extend(pdq)
                    pdq.clear()
            divide(2 * qp + 1)
            while fq:
                fq.popleft()()

        def run(cls):
            for f in cls:
                f()

        # ---- program: A/tr stream (DMA-paced head), C blocks with
        # A/tr/D fillers, out-proj interleaved + short tail.
        run(a_closures(0, 0))
        run(a_closures(1, 0))
        run(a_closures(2, 0))
        run(tr_closures(0, 0, 8))
        c_block(0, 0, fillers=a_closures(2, 1) + tr_closures(0, 8, 16))
        run(a_closures(3, 0))
        run(a_closures(0, 1))
        run(a_closures(1, 1))
        c_block(0, 1, fillers=a_closures(3, 1) + tr_closures(1, 0, 16)
                + a_closures(4, 0))
        c_block(1, 0, fillers=a_closures(4, 1))
        c_block(1, 1, fillers=tr_closures(2, 0, 16))
        c_block(2, 0)
        c_block(2, 1, fillers=d_closures(range(0, 8)))
        # D(8..11) only needs qc2 (done mid-block); its 8 matmuls keep the
        # PE busy while divide(qc3)'s DMA-spread/recip chain percolates,
        # then D(12..15) lands right as oct_ qc3 is written
        run(d_closures(range(8, 16), act_copy=True))

    nc.compile()
    _NC_CACHE["nc"] = nc
    return nc


def make_in_maps(X, Wq, Wk, Wv, Wo):
    X = np.ascontiguousarray(np.asarray(X, dtype=np.float32))
    Wq = np.asarray(Wq, dtype=np.float32)
    Wk = np.asarray(Wk, dtype=np.float32)
    Wv = np.asarray(Wv, dtype=np.float32)
    Wo = np.asarray(Wo, dtype=np.float32)

    # causal additive-bias tiles: keep q >= k; rows=k (p), cols=q (f)
    p = np.arange(128)[:, None]
    f = np.arange(512)[None, :]
    mask = np.where(f[:, :128] >= p, 0.0, MASK_NEG).astype(np.float32)
    ident = np.eye(128, dtype=np.float32).astype(BF)

    in_maps = []
    for c in range(NCORES):
        b, hg = c // 4, c % 4
        gh = [hg * HPC + l for l in range(HPC)]
        q = [Wq[g * DK:(g + 1) * DK, :] / 8.0 for g in gh]
        k = [Wk[g * DK:(g + 1) * DK, :] for g in gh]
        v = [Wv[g * DK:(g + 1) * DK, :] for g in gh]
        wcat_rows = np.vstack([
            q[0], q[1], k[0], k[1], q[2], v[0], k[2], v[1], v[2],
            np.zeros((DK, D), dtype=np.float32),
        ])                                            # (640, 768)
        wcat = np.ascontiguousarray(
            wcat_rows.T.reshape(NI, 128, NM * 128)).astype(BF)
        w0, w1, w2 = (Wo[:, g * DK:(g + 1) * DK].T for g in gh)
        wot = np.ascontiguousarray(np.stack([
            np.vstack([w0, w1]),
            np.vstack([w2, np.zeros((DK, D), dtype=np.float32)]),
        ])).astype(BF)                                # (2, 128, 768)
        xt = np.ascontiguousarray(X[b].T.reshape(NI, 128, S)).astype(BF)
        in_maps.append({
            "xt": xt, "wcat": wcat, "wot": wot,
            "mask": mask, "ident": ident,
        })
    return in_maps


def _run(in_maps, trace=False, trace_cores=None):
    nc = build_nc()
    return bass_utils.run_bass_kernel_spmd(
        nc, in_maps, core_ids=list(range(NCORES)),
        trace=trace, trace_cores=trace_cores,
    )


def kernel(X, Wq, Wk, Wv, Wo):
    in_maps = make_in_maps(X, Wq, Wk, Wv, Wo)
    res = _run(in_maps, trace=False)
    out = np.zeros((B, S, D), dtype=np.float32)
    for c in range(NCORES):
        out[c // 4] += np.asarray(res.results[c]["out"], dtype=np.float32)
    return out


# revision 34
# speedup vs baseline: 1.0209x; 1.0209x over previous
"""Causal multi-head self-attention (B=2, S=2048, D=768, H=12) on 8 TRN2 NeuronCores.

Sharding: core c = (batch b=c//4, head-group hg=c%4 of 3 heads).
Each core computes Q/K/V for its 3 heads, causal attention, and the partial
output projection sum_h out_h @ Wo[:, h]^T -> (S, D). Host sums the 4
head-group partials per batch (the unshard step).

v2 restructure (from trace analysis of the v1 baseline @171us):
  - Input DMA issue is serialized ~730ns per dma_start on the issuing
    engine; v1's 45 fine-grained input DMAs stretched input arrival to
    ~40us. Now ~12 coarse DMAs split across BOTH HWDGE queues (sync +
    scalar), consumption-ordered: input fully lands by ~20us.
  - No warmup chain (v1's 120-mm chain blocked the A stream until 23.8us
    behind a PSUM bank conflict). A starts as soon as wcat m0 + xt sh0/1
    land; the stream itself warms the HAM clock gate while DMA-paced.
  - Single TileContext scope (v1's scope barrier dropped the HAM to
    1.2GHz for 8.2us at the scope-1/2 boundary).
  - Out-projection (D) is split into per-qtile 2x384-col pieces and
    interleaved into the last C block + its tail, so the output DMA
    (issued on sync) overlaps compute instead of forming a 14us tail.
  - A/tr/D work is fed into C-block exp-wait slots as filler closures
    (engine queues are in-order; interleaving must happen at program
    order level).

HAM clock-gate: full 128x128-stationary matmuls everywhere (partial-array
ones don't count as PE-busy and let the clock gate fall to 1.2GHz):
  - scores: stationary = the full 128-row qkvt chunk (the cohabitant
    head's rows are neutralized by a zero-padded Q moving operand qz)
  - PV: stationary = vp = [V | ones | zeros] padded to 128 cols
  - out-proj: oct rows 64-127 of chunk 1 zeroed, stationary full-height
All matmul operands bf16; output written bf16 (host upcasts).

C per head, per q-half qp (2 q-chunks of 512), per k-tile t:
  scoresT[k, q] = Kchunk.T @ qz   (only causally-valid halves)
  additive -30000 mask on the diagonal half, exp on ACT -> bf16,
  PV: pout[qc] += vp[t].T @ expT  (one vp column of ones = denominator;
  h1's V occupies vp cols 64:128 so its numerator lands partition-aligned
  with its oct_ slot)
then per qc: DMA-spread den -> 64-lane recip -> gather -> gpsimd
broadcast -> partition-aligned multiply into outcatT.
D piece: psum[q, j] += outcatT[:, q].T @ WoT[:, j]; copy; DMA out.
"""

import numpy as np
import ml_dtypes
from collections import deque
from contextlib import ExitStack

import concourse.bass as bass
import concourse.tile as tile
from concourse import bacc, mybir
from concourse import bass_utils

F32 = mybir.dt.float32
BF16 = mybir.dt.bfloat16
AF = mybir.ActivationFunctionType
BF = ml_dtypes.bfloat16

B, S, D, H = 2, 2048, 768, 12
DK = 64
HPC = 3            # heads per core
NCORES = 8
NI = D // 128      # 6 input-feature chunks
NM = 5             # output m-chunks of 128 (640 rows incl. 64 pad)
NT = S // 128      # 16 k-tiles
NQC = S // 512     # 4 q-chunks
MASK_NEG = -30000.0

# wcat m-chunks: m0=[q0;q1] m1=[k0;k1] m2=[q2;v0] m3=[k2;v1] m4=[v2;pad]
KCHUNK = [1, 1, 3]   # score stationary = full 128-row chunk holding K_h
VPOS = [(64, 2), (64, 3), (0, 4)]
VCOL = [0, 0, 0]     # V dst col base in vp
NR0 = [0, 0, 0]      # PV numerator psum row base
DROW = [64, 64, 64]  # PV denominator psum row

_NC_CACHE = {}


def build_nc():
    if "nc" in _NC_CACHE:
        return _NC_CACHE["nc"]
    nc = bacc.Bacc("TRN2", target_bir_lowering=False, debug=False,
                   num_devices=NCORES)

    xt_d = nc.dram_tensor("xt", [NI, 128, S], BF16, kind="ExternalInput").ap()
    wcat_d = nc.dram_tensor("wcat", [NI, 128, NM * 128], BF16, kind="ExternalInput").ap()
    wot_d = nc.dram_tensor("wot", [2, 128, D], BF16, kind="ExternalInput").ap()
    mask_d = nc.dram_tensor("mask", [128, 128], F32, kind="ExternalInput").ap()
    id_d = nc.dram_tensor("ident", [128, 128], BF16, kind="ExternalInput").ap()
    out_d = nc.dram_tensor("out", [S, D], BF16, kind="ExternalOutput").ap()

    with tile.TileContext(nc) as tc, ExitStack() as ctx:
        const = ctx.enter_context(tc.tile_pool(name="const", bufs=1))

        # persistent SBUF buffers
        xt = const.tile([128, NI, S], BF16)             # X^T
        wcat = const.tile([128, NI, NM * 128], BF16)    # W^T (QKV packed)
        wot = const.tile([128, 2, D], BF16)             # Wo^T [h0;h1],[h2;0]
        maskb = const.tile([128, 128], F32)             # diag causal bias tile
        ident = const.tile([128, 128], BF16)
        qkvt = const.tile([128, NM, S], BF16)           # K^T/V^T chunks
        qz = const.tile([128, HPC, S], BF16)            # zero-padded Q^T per head
        vp = const.tile([128, HPC, NT, 128], BF16)      # V' = [V | ones | 0]
        oct_ = const.tile([128, 2, S], BF16)            # packed out^T [h0;h1],[h2;0]

        # ---- coarse input DMAs, consumption-ordered, split across the two
        # HWDGE issue queues (sync + scalar) so descriptor pushes (~730ns
        # each, serialized per engine) don't gate data arrival. A single
        # DMA only sustains ~100-150 GB/s, so each xt seq-block is split
        # into two half-depth DMAs, one per queue, to land ~2x faster.
        def dma_wcat(eng, m):
            eng.dma_start(wcat[:, :, m * 128:(m + 1) * 128],
                          wcat_d[:, :, m * 128:(m + 1) * 128]
                          .rearrange("c p f -> p c f"))

        def dma_xt(eng, sh, ih):
            i0, i1 = 3 * ih, 3 * ih + 3
            eng.dma_start(xt[:, i0:i1, sh * 512:(sh + 1) * 512],
                          xt_d[i0:i1, :, sh * 512:(sh + 1) * 512]
                          .rearrange("c p f -> p c f"))

        dma_wcat(nc.sync, 0)
        dma_xt(nc.scalar, 0, 0)
        dma_xt(nc.sync, 0, 1)
        nc.scalar.dma_start(ident[:], id_d)
        dma_xt(nc.sync, 1, 1)
        dma_xt(nc.scalar, 1, 0)
        dma_wcat(nc.sync, 2)
        dma_wcat(nc.scalar, 1)
        nc.sync.dma_start(maskb[:], mask_d)
        dma_xt(nc.scalar, 2, 0)
        dma_xt(nc.sync, 2, 1)
        dma_xt(nc.scalar, 3, 0)
        dma_xt(nc.sync, 3, 1)
        dma_wcat(nc.scalar, 3)
        dma_wcat(nc.sync, 4)
        nc.scalar.dma_start(wot[:], wot_d.rearrange("c p f -> p c f"))

        # zero/one fills on DVE, all disjoint from later writers (the vp pad
        # init deliberately avoids cols 0:DK so the V transposes never wait)
        nc.vector.memzero(qz[64:128, 0, :])
        nc.vector.memzero(qz[0:64, 1, :])
        nc.vector.memzero(qz[64:128, 2, :])
        nc.vector.memzero(oct_[64:128, 1, :])
        nc.vector.memzero(vp[:, :, :, DK:128])         # pad cols
        nc.vector.memset(vp[:, :, :, DK:DK + 1], 1.0)  # denominator ones col


        sb_exp = ctx.enter_context(tc.tile_pool(name="sb_exp", bufs=6))
        sb_div = ctx.enter_context(tc.tile_pool(name="sb_div", bufs=3))

        ps_f = ctx.enter_context(tc.tile_pool(name="ps_f", bufs=2, space="PSUM"))
        ps_s = ctx.enter_context(tc.tile_pool(name="ps_s", bufs=2, space="PSUM"))
        ps_o = ctx.enter_context(tc.tile_pool(name="ps_o", bufs=2, space="PSUM"))

        def a_closures(m, scp):
            """QKV projection for m-chunk m, seq half scp (2x512 cols), as
            one closure per contraction chunk i (2 matmuls each)."""
            pqs = [None, None]

            def step(i):
                for half in range(2):
                    sc = 2 * scp + half
                    if i == 0:
                        pqs[half] = ps_f.tile([128, 512], F32, tag="fill",
                                              name=f"pq{m}_{2 * scp + half}")
                    nc.tensor.matmul(
                        pqs[half][:],
                        wcat[:, i, m * 128:(m + 1) * 128],
                        xt[:, i, sc * 512:(sc + 1) * 512],
                        start=(i == 0), stop=(i == NI - 1))
                if i != NI - 1:
                    return
                for half in range(2):
                    sc = 2 * scp + half
                    s0, s1 = sc * 512, (sc + 1) * 512
                    pq = pqs[half]
                    if m == 0:      # pure Q chunk -> zero-padded q shadows
                        nc.vector.tensor_copy(qz[0:64, 0, s0:s1], pq[0:64, :])
                        nc.vector.tensor_copy(qz[64:128, 1, s0:s1], pq[64:128, :])
                    elif m == 2:    # [q2; v0]
                        nc.vector.tensor_copy(qz[0:64, 2, s0:s1], pq[0:64, :])
                        nc.vector.tensor_copy(qkvt[64:128, m, s0:s1], pq[64:128, :])
                    elif m == 4:    # [v2; pad]
                        nc.vector.tensor_copy(qkvt[0:64, m, s0:s1], pq[0:64, :])
                    else:           # full K chunks (score stationaries)
                        nc.vector.tensor_copy(qkvt[:, m, s0:s1], pq[:])

            return [(lambda i=i: step(i)) for i in range(NI)]

        def tr_closures(h, tlo, thi):
            """V transposes into vp, 4 k-tiles per closure."""
            vb, vchunk = VPOS[h]
            vc = VCOL[h]

            def step(base):
                for t in range(base, min(base + 4, thi)):
                    ptr = ps_f.tile([128, DK], BF16, tag="fill",
                                    name=f"tr{h}_{t}")
                    nc.tensor.transpose(
                        ptr[:], qkvt[vb:vb + DK, vchunk, t * 128:(t + 1) * 128],
                        ident[vb:vb + DK, vb:vb + DK])
                    nc.vector.tensor_copy(vp[:, h, t, vc:vc + DK], ptr[:])

            return [(lambda b=b: step(b)) for b in range(tlo, thi, 4)]

        def d_closures(qts, act_copy=False):
            """Out-projection pieces: per q-tile, 2 closures of a 2x384-col
            matmul pair + copy; the second closure DMAs the tile out."""
            cls = []
            for qt in qts:
                oref = [None]

                def piece(qt, p, oref):
                    if p == 0:
                        oref[0] = sb_exp.tile([128, D], BF16, tag="exp",
                                              name=f"ot{qt}")
                    pp = ps_f.tile([128, 384], F32, tag="fill",
                                   name=f"pp{qt}_{p}")
                    for c in (0, 1):
                        nc.tensor.matmul(
                            pp[:],
                            oct_[:, c, qt * 128:(qt + 1) * 128],
                            wot[:, c, p * 384:(p + 1) * 384],
                            start=(c == 0), stop=(c == 1))
                    if act_copy:
                        nc.scalar.copy(oref[0][:, p * 384:(p + 1) * 384], pp[:])
                    else:
                        nc.vector.tensor_copy(
                            oref[0][:, p * 384:(p + 1) * 384], pp[:])
                    if p == 1:
                        nc.sync.dma_start(
                            out_d[qt * 128:(qt + 1) * 128, :], oref[0][:])

                cls.append(lambda qt=qt, o=oref: piece(qt, 0, o))
                cls.append(lambda qt=qt, o=oref: piece(qt, 1, o))
            return cls

        def c_block(h, qp, fillers=(), post_div_fillers=()):
            kchunk = KCHUNK[h]
            pouts = {}
            fq = deque(fillers)
            pdq = deque(post_div_fillers)

            def score_step(t):
                qcs = (2 * qp, 2 * qp + 1)
                qc_lo = t // 4
                off = 128 * (t % 4)   # diag col offset inside qc_lo's half
                pscr = ps_s.tile([128, 1024], F32, tag="scr",
                                 name=f"sc{h}_{qp}_{t}")
                for half, qc in enumerate(qcs):
                    if qc < qc_lo:
                        continue
                    cs = off if qc == qc_lo else 0  # skip fully-masked cols
                    nc.tensor.matmul(
                        pscr[:, half * 512 + cs:(half + 1) * 512],
                        qkvt[:, kchunk, t * 128:(t + 1) * 128],
                        qz[:, h, qc * 512 + cs:(qc + 1) * 512],
                        start=True, stop=True)
                if qc_lo in qcs:  # mask only the 128-wide diagonal window
                    half = qc_lo - 2 * qp
                    nc.vector.tensor_add(
                        pscr[:, half * 512 + off:half * 512 + off + 128],
                        pscr[:, half * 512 + off:half * 512 + off + 128],
                        maskb[:, 0:128])
                lo = (512 if qc_lo == qcs[1] else 0) + \
                     (off if qc_lo in qcs else 0)
                expt = sb_exp.tile([128, 1024], BF16, tag="exp",
                                   name=f"ex{h}_{qp}_{t}")
                nc.scalar.activation(expt[:, lo:1024], pscr[:, lo:1024],
                                     AF.Exp)
                return expt

            def pv_step(t, expt):
                qcs = (2 * qp, 2 * qp + 1)
                qc_lo = t // 4
                off = 128 * (t % 4)
                for half, qc in enumerate(qcs):
                    if qc < qc_lo:
                        continue
                    cs = off if qc == qc_lo else 0
                    nc.tensor.matmul(
                        pouts[qc][:, cs:512],
                        vp[:, h, t, :],
                        expt[:, half * 512 + cs:(half + 1) * 512],
                        start=(t == 0), stop=(t == 4 * qc + 3))

            def divide(qc):
                # evict the finished chain so its PSUM bank frees; the
                # recip runs 64 lanes wide on a DMA-spread [64,8] view of
                # the den row (a [64,512]-wide reciprocal is ~6.5ns/elem on
                # DVE -- measured 3.3us -- so keep recip elements minimal).
                nout = sb_div.tile([DK + 1, 512], F32, tag="nout",
                                   name=f"no{h}_{qc}")
                nc.vector.tensor_copy(nout[:], pouts[qc][0:DK + 1, :])
                rsp = sb_div.tile([DK, 8], F32, tag="rsp", name=f"rsp{h}_{qc}")
                nc.sync.dma_start(rsp[:], nout[DK:DK + 1, :])
                rcs = sb_div.tile([DK, 8], F32, tag="rcs", name=f"rcs{h}_{qc}")
                nc.vector.reciprocal(rcs[:], rsp[:])
                rc0 = sb_div.tile([1, 512], F32, tag="rc0", name=f"rc0{h}_{qc}")
                nc.sync.dma_start(rc0[:], rcs[:])
                rb = sb_div.tile([DK, 512], F32, tag="rb", name=f"rb{h}_{qc}")
                nc.gpsimd.partition_broadcast(rb[:], rc0[:])
                if h == 1:
                    # h1 lands at partitions 64-127: shift via SBUF DMA
                    tmp = sb_div.tile([DK, 512], BF16, tag="tmp",
                                      name=f"tmp{h}_{qc}")
                    nc.vector.tensor_mul(tmp[:], nout[0:DK, :], rb[:])
                    nc.sync.dma_start(
                        oct_[DK:128, 0, qc * 512:(qc + 1) * 512], tmp[:])
                else:
                    nc.vector.tensor_mul(
                        oct_[0:DK, h // 2, qc * 512:(qc + 1) * 512],
                        nout[0:DK, :], rb[:])

            for qc in (2 * qp, 2 * qp + 1):
                pouts[qc] = ps_o.tile([128, 512], F32, tag="pout",
                                      name=f"po{h}_{qc}")
            t_pairs = list(range(0, 4 * (2 * qp + 1) + 4, 2))
            for pi, t0 in enumerate(t_pairs):
                e0 = score_step(t0)
                e1 = score_step(t0 + 1)
                # feed independent work into the exp-wait slot
                iters_left = len(t_pairs) - pi
                for _ in range(-(-len(fq) // iters_left)):
                    if fq:
                        fq.popleft()()
                pv_step(t0, e0)
                pv_step(t0 + 1, e1)
                if t0 + 1 == 4 * (2 * qp) + 3:
                    divide(2 * qp)      # low chain done: free its bank
                    fq.extend(pdq)
                    pdq.clear()
            divide(2 * qp + 1)
            while fq:
                fq.popleft()()

        def run(cls):
            for f in cls:
                f()

        # ---- program: A/tr stream (DMA-paced head), C blocks with
        # A/tr/D fillers, out-proj interleaved + short tail.
        run(a_closures(0, 0))
        run(a_closures(1, 0))
        run(a_closures(2, 0))
        run(tr_closures(0, 0, 8))
        c_block(0, 0, fillers=a_closures(2, 1) + tr_closures(0, 8, 16))
        run(a_closures(3, 0))
        run(a_closures(0, 1))
        run(a_closures(1, 1))
        c_block(0, 1, fillers=a_closures(3, 1) + tr_closures(1, 0, 16)
                + a_closures(4, 0))
        c_block(1, 0, fillers=a_closures(4, 1))
        c_block(1, 1, fillers=tr_closures(2, 0, 16))
        c_block(2, 0)
        c_block(2, 1, fillers=d_closures(range(0, 8)))
        # D(8..11) only needs qc2 (done mid-block); its 8 matmuls keep the
        # PE busy while divide(qc3)'s DMA-spread/recip chain percolates,
        # then D(12..15) lands right as oct_ qc3 is written
        run(d_closures(range(8, 16), act_copy=True))

    nc.compile()
    _NC_CACHE["nc"] = nc
    return nc


def make_in_maps(X, Wq, Wk, Wv, Wo):
    X = np.ascontiguousarray(np.asarray(X, dtype=np.float32))
    Wq = np.asarray(Wq, dtype=np.float32)
    Wk = np.asarray(Wk, dtype=np.float32)
    Wv = np.asarray(Wv, dtype=np.float32)
    Wo = np.asarray(Wo, dtype=np.float32)

    # causal additive-bias tiles: keep q >= k; rows=k (p), cols=q (f)
    p = np.arange(128)[:, None]
    f = np.arange(512)[None, :]
    mask = np.where(f[:, :128] >= p, 0.0, MASK_NEG).astype(np.float32)
    ident = np.eye(128, dtype=np.float32).astype(BF)

    in_maps = []
    for c in range(NCORES):
        b, hg = c // 4, c % 4
        gh = [hg * HPC + l for l in range(HPC)]
        q = [Wq[g * DK:(g + 1) * DK, :] / 8.0 for g in gh]
        k = [Wk[g * DK:(g + 1) * DK, :] for g in gh]
        v = [Wv[g * DK:(g + 1) * DK, :] for g in gh]
        wcat_rows = np.vstack([
            q[0], q[1], k[0], k[1], q[2], v[0], k[2], v[1], v[2],
            np.zeros((DK, D), dtype=np.float32),
        ])                                            # (640, 768)
        wcat = np.ascontiguousarray(
            wcat_rows.T.reshape(NI, 128, NM * 128)).astype(BF)
        w0, w1, w2 = (Wo[:, g * DK:(g + 1) * DK].T for g in gh)
        wot = np.ascontiguousarray(np.stack([
            np.vstack([w0, w1]),
            np.vstack([w2, np.zeros((DK, D), dtype=np.float32)]),
        ])).astype(BF)                                # (2, 128, 768)
        xt = np.ascontiguousarray(X[b].T.reshape(NI, 128, S)).astype(BF)
        in_maps.append({
            "xt": xt, "wcat": wcat, "wot": wot,
            "mask": mask, "ident": ident,
        })
    return in_maps


def _run(in_maps, trace=False, trace_cores=None):
    nc = build_nc()
    return bass_utils.run_bass_kernel_spmd(
        nc, in_maps, core_ids=list(range(NCORES)),
        trace=trace, trace_cores=trace_cores,
    )


def kernel(X, Wq, Wk, Wv, Wo):
    in_maps = make_in_maps(X, Wq, Wk, Wv, Wo)
    res = _run(in_maps, trace=False)
    out = np.zeros((B, S, D), dtype=np.float32)
    for c in range(NCORES):
        out[c // 4] += np.asarray(res.results[c]["out"], dtype=np.float32)
    return out
